# revision 1
# baseline (speedup 1.0000x reference)
"""BallQuery Trainium2 kernel.

Problem: xyz (8, 8192, 3) f32, new_xyz (8, 2048, 3) f32 -> out (8, 2048, 32) int32.
For each query row (b, m): the first 32 point indices j (ascending) with
|q - p_j|^2 < 0.1^2, padded with the first valid index; all-sentinel (8193)
when no point is in radius.

Sharding: data-parallel over batch — core i handles batch i (8 cores).

Exactness: the reference (jax CPU) computes f32 d_k = q_k - p_k, f32 squares,
and the f32 sum ((dx^2+dy^2)+dz^2) compared < r^2.  This kernel replicates
that exact rounding:
  - ACT engine: sq_k = Square(1.0*p_k + (-q_k))   (exact f32 affine + square)
  - DVE: a1 = sqx+sqy ; a2 = sqz+a1 (f32 add is commutative-exact) ;
         mask = a2 < r2 (exact compare)
Selection: running clamped count via DVE tensor_tensor_scan
(state = min(state + mask, 32), initial -1) written REVERSED as int16 ->
per-element scatter slot; GPSIMD local_scatter writes (j+1-32768) to slot
rank-1, iterating descending j so the smallest j wins each slot; min-merge
across chunks; small finalize applies the reference's padding semantics.
"""

import numpy as np

import concourse.bacc as bacc
import concourse.bass as bass
import concourse.mybir as mybir
from concourse import bass_utils
from concourse.tile import TileContext

B, N, M, NS = 8, 8192, 2048, 32
RADIUS2 = np.float32(0.1) * np.float32(0.1)
SENT = N + 1  # 8193, reference sentinel
QTR = N // 4   # 2048: n processed in four quarters (SBUF budget)
CHUNK = 1024   # local_scatter chunk
NSLOT = 34     # scatter dst slots: ranks 0..31 + trash 32 (+pad to even)
NT = M // 128  # 16 m-tiles
OFF = 32768    # int16 offset so scattered values are negative (0 = empty)

_PLAN = {}


def _build():
    if "nc" in _PLAN:
        return _PLAN["nc"]
    f32 = mybir.dt.float32
    bf16 = mybir.dt.bfloat16
    i16 = mybir.dt.int16
    i32 = mybir.dt.int32
    Alu = mybir.AluOpType
    Act = mybir.ActivationFunctionType

    nc = bacc.Bacc("TRN2", target_bir_lowering=False)
    xyz_t = nc.dram_tensor("xyz_b", [N, 3], f32, kind="ExternalInput")
    new_t = nc.dram_tensor("new_b", [M, 3], f32, kind="ExternalInput")
    out_t = nc.dram_tensor("out_b", [M, NS], i32, kind="ExternalOutput")
    pk_dram = nc.dram_tensor("pk_scratch", [3, N], f32)

    # Scatter data constants: value at reversed position p (quarter h) is
    # j + 1 - OFF with j = h*QTR + (QTR-1) - p.
    descs = []
    for h in range(4):
        row = (h * QTR + QTR - np.arange(QTR, dtype=np.int64) - OFF).astype(
            np.int16
        )
        descs.append(np.ascontiguousarray(np.broadcast_to(row, (128, QTR))))
    desc_d = [nc.inline_tensor(d, name=f"desc{h}") for h, d in enumerate(descs)]

    with TileContext(nc) as tc:
        with (
            tc.tile_pool(name="const", bufs=1) as cpool,
            tc.tile_pool(name="rep", bufs=2) as rpool,
            tc.tile_pool(name="sq", bufs=2) as sqpool,
            tc.tile_pool(name="mask", bufs=2) as mpool,
            tc.tile_pool(name="idx", bufs=2) as ipool,
            tc.tile_pool(name="fin", bufs=2) as fpool,
        ):
            # --- one-time setup ---
            with nc.allow_non_contiguous_dma(
                reason="one-time 98KB coord-split gather of xyz to DRAM scratch"
            ):
                nc.sync.dma_start(pk_dram[:], xyz_t[:].rearrange("n c -> c n"))

            q_tile = cpool.tile([128, NT * 3], f32)
            nc.sync.dma_start(
                q_tile[:, :].rearrange("p (t c) -> p t c", c=3),
                new_t[:].rearrange("(t p) c -> p t c", p=128),
            )
            negq = cpool.tile([128, NT * 3], f32)
            nc.vector.tensor_scalar(negq, q_tile, -1.0, None, Alu.mult)

            desc_s = []
            for h in range(4):
                d = cpool.tile([128, QTR], i16, tag=f"desc{h}")
                nc.sync.dma_start(d[:, :], desc_d[h][:])
                desc_s.append(d)

            c32 = cpool.tile([128, 1], bf16)
            nc.vector.memset(c32, 32.0)
            negr2 = cpool.tile([128, 1], f32)
            nc.vector.memset(negr2, -float(RADIUS2))

            # per-(tile) scatter outputs: 8 chunks x NSLOT, persistent
            dst_all = cpool.tile([128, NT * 8 * NSLOT], i16)
            carry = cpool.tile([128, NT], i16)

            # --- main pipeline ---
            for h in range(4):
                rep = []
                for k in range(3):
                    r = rpool.tile([128, QTR], f32, tag=f"rep{k}")
                    src_ap = pk_dram[k, h * QTR : (h + 1) * QTR]
                    nc.sync.dma_start(r[:, :], src_ap.partition_broadcast(128))
                    rep.append(r)

                for t in range(NT):
                    mask_h = mpool.tile([128, QTR], bf16)
                    sq = []
                    for k in range(3):
                        s = sqpool.tile([128, QTR], f32, tag=f"sq{k}")
                        nc.scalar.activation(
                            s[:, :],
                            rep[k][:, :],
                            Act.Square,
                            bias=negq[:, t * 3 + k : t * 3 + k + 1],
                            scale=1.0,
                        )
                        sq.append(s)
                    # a1 = sqx + sqy (in sq[0]); a2 = sqz + a1 (in sq[2])
                    nc.vector.tensor_add(sq[0], sq[0], sq[1])
                    nc.vector.tensor_add(sq[2], sq[2], sq[0])
                    # mask on GPSIMD (InstTensorScalarPtr is builtin ucode --
                    # no library conflict with local_scatter)
                    nc.gpsimd.tensor_scalar(
                        mask_h[:, :], sq[2], float(RADIUS2), None, Alu.is_lt
                    )

                    idxrev = ipool.tile([128, QTR], i16)
                    initial = -1.0 if h == 0 else carry[:, t : t + 1]
                    nc.vector.tensor_tensor_scan(
                        idxrev[:, ::-1],
                        mask_h[:, :],
                        c32.to_broadcast([128, QTR]),
                        initial,
                        Alu.add,
                        Alu.min,
                    )
                    if h < 3:
                        nc.gpsimd.tensor_scalar(
                            carry[:, t : t + 1], idxrev[:, 0:1], 0.0, None, Alu.add
                        )

                    for c in range(QTR // CHUNK):
                        sl = slice(c * CHUNK, (c + 1) * CHUNK)
                        di = (t * 8 + h * 2 + c) * NSLOT
                        nc.gpsimd.local_scatter(
                            dst_all[:, di : di + NSLOT],
                            desc_s[h][:, sl],
                            idxrev[:, sl],
                            channels=128,
                            num_elems=NSLOT,
                            num_idxs=CHUNK,
                        )

            # --- batched merge + finalize (strided APs over all 16 tiles) ---
            # dst_all viewed as [128, NT, 8, NSLOT]; min-tree over the 8 chunks
            d4 = dst_all[:, :].rearrange("p (t c s) -> p t c s", c=8, s=NSLOT)
            m4 = fpool.tile([128, NT * 4 * NSLOT], i16, tag="m4")
            m4v = m4[:, :].rearrange("p (t c s) -> p t c s", c=4, s=NSLOT)
            nc.vector.tensor_tensor(
                out=m4v, in0=d4[:, :, 0:4, :], in1=d4[:, :, 4:8, :], op=Alu.min
            )
            m2 = fpool.tile([128, NT * 2 * NSLOT], i16, tag="m2")
            m2v = m2[:, :].rearrange("p (t c s) -> p t c s", c=2, s=NSLOT)
            nc.vector.tensor_tensor(
                out=m2v, in0=m4v[:, :, 0:2, :], in1=m4v[:, :, 2:4, :], op=Alu.min
            )
            mg = fpool.tile([128, NT * NSLOT], i16, tag="mg")
            mgv = mg[:, :].rearrange("p (t s) -> p t s", s=NSLOT)
            nc.vector.tensor_tensor(
                out=mgv, in0=m2v[:, :, 0, :], in1=m2v[:, :, 1, :], op=Alu.min
            )

            # v = merged[:, :, :32] + (OFF-1): j for valid slots, 32767 empty
            v = fpool.tile([128, NT * NS], f32, tag="v")
            vv = v[:, :].rearrange("p (t s) -> p t s", s=NS)
            nc.gpsimd.tensor_scalar(
                vv, mgv[:, :, :NS], float(OFF - 1), None, Alu.add
            )
            e = fpool.tile([128, NT * NS], f32, tag="e")
            ev = e[:, :].rearrange("p (t s) -> p t s", s=NS)
            nc.gpsimd.tensor_scalar(ev, vv, float(OFF - 1), None, Alu.is_equal)
            a = fpool.tile([128, NT], f32, tag="a")
            nc.gpsimd.tensor_scalar(
                a, vv[:, :, 0], float(OFF - 1), None, Alu.is_equal
            )
            fs = fpool.tile([128, NT], f32, tag="fs")
            nc.vector.scalar_tensor_tensor(
                out=fs,
                in0=a,
                scalar=float(SENT - (OFF - 1)),
                in1=vv[:, :, 0],
                op0=Alu.mult,
                op1=Alu.add,
            )
            # u1 = v - fs (fs broadcast along slots); u2 = e*u1; out = v - u2
            u1 = fpool.tile([128, NT * NS], f32, tag="u1")
            u1v = u1[:, :].rearrange("p (t s) -> p t s", s=NS)
            nc.vector.tensor_tensor(
                out=u1v,
                in0=vv,
                in1=fs[:, :].to_broadcast([128, NT, NS]),
                op=Alu.subtract,
            )
            u2 = fpool.tile([128, NT * NS], f32, tag="u2")
            u2v = u2[:, :].rearrange("p (t s) -> p t s", s=NS)
            nc.vector.tensor_tensor(out=u2v, in0=ev, in1=u1v, op=Alu.mult)
            o32 = fpool.tile([128, NT * NS], i32, tag="o32")
            o32v = o32[:, :].rearrange("p (t s) -> p t s", s=NS)
            nc.vector.tensor_tensor(out=o32v, in0=vv, in1=u2v, op=Alu.subtract)

            nc.sync.dma_start(
                out_t[:].rearrange("(t p) s -> p t s", p=128), o32v
            )

    nc.compile()
    _PLAN["nc"] = nc
    return nc


def kernel(xyz: np.ndarray, new_xyz: np.ndarray) -> np.ndarray:
    xyz = np.ascontiguousarray(np.asarray(xyz, dtype=np.float32))
    new_xyz = np.ascontiguousarray(np.asarray(new_xyz, dtype=np.float32))
    nc = _build()
    in_maps = [
        {"xyz_b": xyz[b], "new_b": new_xyz[b]} for b in range(B)
    ]
    res = bass_utils.run_bass_kernel_spmd(nc, in_maps, core_ids=list(range(B)))
    return np.stack([res.results[b]["out_b"] for b in range(B)], axis=0).astype(
        np.int32
    )


if __name__ == "__main__":
    rng = np.random.default_rng(0)
    x = rng.random((B, N, 3), dtype=np.float32)
    q = rng.random((B, M, 3), dtype=np.float32)
    out = kernel(x, q)
    print(out.shape, out.dtype)



# revision 2
# speedup vs baseline: 2.6606x; 2.6606x over previous
"""BallQuery Trainium2 kernel, v2 (PE-matmul + pair-compressed selection).

Problem: xyz (8, 8192, 3) f32, new_xyz (8, 2048, 3) f32 -> out (8, 2048, 32)
int32.  For each query row (b, m): the first 32 point indices j (ascending)
with |q - p_j|^2 < 0.1^2 (f32 reference rounding), padded with the first
valid index; all-sentinel (8193) when none.

Sharding: data-parallel over batch - core b handles batch b (8 cores).

Device algorithm (per core):
  - Host prepares (coords shifted by -0.5 to halve fp32r magnitudes):
      pmat[5, N] = [px, py, pz, |p|^2, 1]
      qmat[5, M] = [-2qx, -2qy, -2qz, 1, |q|^2 - r^2 - EPS]
    so PE fp32r matmul gives psum = |q-p|^2 - r^2 - EPS (error <~ 5e-4,
    EPS = 1.8e-3 makes the device mask a strict SUPERSET of the reference
    mask: no false negatives).
  - ACT Sign(-psum) (or DVE is_lt on a few tiles for balance) -> per-point
    inclusion in bf16.
  - One DVE tensor_scalar not_equal on the uint32-punned bf16 PAIR decides
    "either point of the pair is in" (2x_2p mode): (-1,-1) <-> 0xBF80BF80
    for the sign path, (0,0) <-> 0x0 for the is_lt path.
  - DVE reversed prefix scan (4x mode; 64 zero-pads soak up the DVE
    pipeline-warmup glitch) ranks valid pairs, clamped at 64, carried
    across the four n-quarters.
  - GPSIMD local_scatter writes (pairidx+1-32768) to slot rank, iterating
    descending pairidx so the smallest wins; DVE min-merges the 4 quarter
    outputs; slots DMA out as int16 [M, 66].
Host decodes: up to 2 candidate points per filled pair slot, re-checks all
candidates with exact reference f32 rounding, takes the first 32, applies
reference padding.  Rows that overflowed the 64-pair pool AND came up short
are recomputed exactly on host (expected ~0 rows).
"""

import numpy as np

import concourse.bacc as bacc
import concourse.bass as bass
import concourse.mybir as mybir
from concourse import bass_utils
from concourse.tile import TileContext

B, N, M = 8, 8192, 2048
NS = 32
K = 5
QTR = 2048
NT = M // 128  # 16 m-tiles
PAIRS_Q = QTR // 2  # 1024 pairs per n-quarter
PAD = 64  # zero pad processed first by the scan (warmup protection)
CLAMP = 64  # pair-rank clamp = candidate pool size
NSLOT = 66  # slots: ranks 0..63 + trash 64 + pad 65 (even)
OFF = 32768
SENT = N + 1
RADIUS2 = np.float32(0.1) * np.float32(0.1)
EPS = np.float32(1.8e-3)  # superset margin > max fp32r deviation (~5e-4)
PUN_SIGN = float(0xBF80BF80)  # bf16 pair (-1.0, -1.0): both out (sign path)
PUN_MASK = 0.0  # bf16 pair (0.0, 0.0): both out (is_lt path)
BIG = 1 << 30

_PLAN = {}


def _build():
    if "nc" in _PLAN:
        return _PLAN["nc"]
    f32 = mybir.dt.float32
    f32r = mybir.dt.float32r
    bf16 = mybir.dt.bfloat16
    i16 = mybir.dt.int16
    u32 = mybir.dt.uint32
    Alu = mybir.AluOpType
    Act = mybir.ActivationFunctionType

    nc = bacc.Bacc("TRN2", target_bir_lowering=False)
    qm_t = nc.dram_tensor("qmat", [K, M], f32r, kind="ExternalInput")
    pm_t = nc.dram_tensor("pmat", [K, N], f32r, kind="ExternalInput")
    out_t = nc.dram_tensor("slots", [M, NSLOT], i16, kind="ExternalOutput")

    # pair descriptors, mirrored: value at scan-output position p of quarter
    # h is pairidx+1-OFF with pairidx = h*PAIRS_Q + (PAIRS_Q-1-p).
    descs = []
    for h in range(4):
        base = h * PAIRS_Q
        d = (base + PAIRS_Q - np.arange(PAIRS_Q, dtype=np.int64) - OFF).astype(
            np.int16
        )
        descs.append(np.ascontiguousarray(np.broadcast_to(d, (128, PAIRS_Q))))
    desc_d = [nc.inline_tensor(d, name=f"desc{h}") for h, d in enumerate(descs)]

    with TileContext(nc) as tc:
        with (
            tc.tile_pool(name="const", bufs=1) as cpool,
            tc.tile_pool(name="sgn", bufs=2) as spool,
            tc.tile_pool(name="pmx", bufs=2) as xpool,
            tc.tile_pool(name="scan", bufs=2) as ipool,
            tc.psum_pool(name="ps", bufs=2) as pp,
        ):
            qt = cpool.tile([K, M], f32r)
            nc.sync.dma_start(qt[:, :], qm_t[:])
            pt = cpool.tile([K, N], f32r)
            nc.sync.dma_start(pt[:, :], pm_t[:])
            desc_s = []
            for h in range(4):
                d = cpool.tile([128, PAIRS_Q], i16, tag=f"desc{h}")
                nc.sync.dma_start(d[:, :], desc_d[h][:])
                desc_s.append(d)
            cC = cpool.tile([128, PAD + PAIRS_Q], bf16)
            nc.vector.memset(cC, float(CLAMP))

            dsts = cpool.tile([128, NT * 4 * NSLOT], i16)

            for t in range(NT):
                prev_scan = None
                for h in range(4):
                    ps = pp.tile([128, QTR], f32, tag="ps")
                    for c in range(QTR // 512):
                        nc.tensor.matmul(
                            ps[:, c * 512 : (c + 1) * 512],
                            qt[:, t * 128 : (t + 1) * 128],
                            pt[:, h * QTR + c * 512 : h * QTR + (c + 1) * 512],
                        )

                    s_t = spool.tile([128, QTR], bf16, tag="sgn")
                    # 12 of 64 units on DVE to balance ACT/DVE load
                    if h == 3 and t % 4 != 3:
                        nc.vector.tensor_scalar(s_t, ps, 0.0, None, Alu.is_lt)
                        pun_c = PUN_MASK
                    else:
                        nc.scalar.activation(
                            s_t[:, :], ps[:, :], Act.Sign, bias=0.0, scale=-1.0
                        )
                        pun_c = PUN_SIGN

                    pmx = xpool.tile([128, PAD + PAIRS_Q], bf16, tag="pmx")
                    if t == 0 and h < 2:
                        # zero the warmup pad once per ring buffer; later
                        # iterations only write [PAD:] so it stays zero.
                        nc.vector.memset(pmx[:, 0:PAD], 0.0)
                    nc.vector.tensor_scalar(
                        pmx[:, PAD:], s_t[:, :].bitcast(u32), pun_c, None,
                        Alu.not_equal,
                    )

                    sc = ipool.tile([128, PAD + PAIRS_Q], i16, tag="scan")
                    initial = -1.0 if h == 0 else prev_scan[:, 0:1]
                    nc.vector.tensor_tensor_scan(
                        sc[:, ::-1], pmx[:, :], cC[:, :], initial,
                        Alu.add, Alu.min,
                    )
                    prev_scan = sc

                    nc.gpsimd.local_scatter(
                        dsts[:, (t * 4 + h) * NSLOT : (t * 4 + h + 1) * NSLOT],
                        desc_s[h][:, :],
                        sc[:, 0:PAIRS_Q],
                        channels=128,
                        num_elems=NSLOT,
                        num_idxs=PAIRS_Q,
                    )

            # min-merge the 4 quarters per m-tile (empty slots are 0; filled
            # are negative, so min picks filled and smallest pairidx).
            d4 = dsts[:, :].rearrange("p (t c s) -> p t c s", c=4, s=NSLOT)
            m2 = cpool.tile([128, NT * 2 * NSLOT], i16)
            m2v = m2[:, :].rearrange("p (t c s) -> p t c s", c=2, s=NSLOT)
            nc.vector.tensor_tensor(
                out=m2v, in0=d4[:, :, 0:2, :], in1=d4[:, :, 2:4, :], op=Alu.min
            )
            mg = cpool.tile([128, NT * NSLOT], i16)
            mgv = mg[:, :].rearrange("p (t s) -> p t s", s=NSLOT)
            nc.vector.tensor_tensor(
                out=mgv, in0=m2v[:, :, 0, :], in1=m2v[:, :, 1, :], op=Alu.min
            )
            nc.sync.dma_start(
                out_t[:].rearrange("(t p) s -> p t s", p=128), mgv
            )

    nc.compile()
    _PLAN["nc"] = nc
    return nc


def _prep(xyz_b: np.ndarray, new_b: np.ndarray):
    half = np.float32(0.5)
    ps = (xyz_b - half).astype(np.float32)
    qs = (new_b - half).astype(np.float32)
    pmat = np.zeros((K, N), dtype=np.float32)
    pmat[0:3] = ps.T
    pmat[3] = (ps * ps).sum(1, dtype=np.float32)
    pmat[4] = 1.0
    qmat = np.zeros((K, M), dtype=np.float32)
    qmat[0:3] = (np.float32(-2.0) * qs).T
    qmat[3] = 1.0
    qmat[4] = (qs * qs).sum(1, dtype=np.float32) - RADIUS2 - EPS
    return pmat, qmat


def _ref_row(q: np.ndarray, pts: np.ndarray) -> np.ndarray:
    """Exact reference semantics for one query row (f32 rounding)."""
    d = (q[None, :] - pts).astype(np.float32)
    sq = (d * d).astype(np.float32)
    s2 = ((sq[:, 0] + sq[:, 1]) + sq[:, 2]).astype(np.float32)
    idx = np.where(s2 < RADIUS2)[0]
    out = np.full(NS, SENT, dtype=np.int64)
    k = min(len(idx), NS)
    out[:k] = idx[:NS]
    if k:
        out[k:] = idx[0]
    return out


def kernel(xyz: np.ndarray, new_xyz: np.ndarray) -> np.ndarray:
    xyz = np.ascontiguousarray(np.asarray(xyz, dtype=np.float32))
    new_xyz = np.ascontiguousarray(np.asarray(new_xyz, dtype=np.float32))
    nc = _build()
    in_maps = []
    for b in range(B):
        pmat, qmat = _prep(xyz[b], new_xyz[b])
        in_maps.append({"pmat": pmat, "qmat": qmat})
    res = bass_utils.run_bass_kernel_spmd(nc, in_maps, core_ids=list(range(B)))
    slots = np.stack(
        [res.results[b]["slots"] for b in range(B)], axis=0
    )  # [B, M, NSLOT] int16

    pool = slots[:, :, :CLAMP].astype(np.int64)
    filled = pool != 0
    pairidx = np.where(filled, pool + (OFF - 1), 0)  # [B, M, CLAMP]
    cand = np.empty((B, M, CLAMP, 2), dtype=np.int64)
    cand[..., 0] = 2 * pairidx
    cand[..., 1] = 2 * pairidx + 1
    bidx = np.arange(B)[:, None, None, None]
    gat = xyz[bidx, cand, :]  # [B, M, CLAMP, 2, 3]
    d = (new_xyz[:, :, None, None, :] - gat).astype(np.float32)
    sq = (d * d).astype(np.float32)
    s2 = ((sq[..., 0] + sq[..., 1]) + sq[..., 2]).astype(np.float32)
    keep = filled[..., None] & (s2 < RADIUS2)  # [B, M, CLAMP, 2]

    candf = cand.reshape(B, M, 2 * CLAMP)
    keepf = keep.reshape(B, M, 2 * CLAMP)
    masked = np.where(keepf, candf, BIG)
    svals = np.sort(masked, axis=2)[:, :, :NS]
    vals = np.where(svals >= BIG, SENT, svals)
    first = vals[:, :, 0:1]
    out = np.where(vals == SENT, first, vals)

    # pool-overflow fixup: row hit the 64-pair clamp AND kept < 32
    kept_count = keepf.sum(axis=2)
    trash = slots[:, :, CLAMP] != 0
    bad = (kept_count < NS) & trash
    for b, m in np.argwhere(bad):
        out[b, m] = _ref_row(new_xyz[b, m], xyz[b])

    return out.astype(np.int32)


if __name__ == "__main__":
    rng = np.random.default_rng(0)
    x = rng.random((B, N, 3), dtype=np.float32)
    q = rng.random((B, M, 3), dtype=np.float32)
    o = kernel(x, q)
    print(o.shape, o.dtype)


# revision 6
# speedup vs baseline: 6.2310x; 2.3420x over previous
"""BallQuery Trainium2 kernel, v3: x-sorted spatial windows + PE fp32r
distance matmul + quad-compressed candidate extraction.

Problem: xyz (8, 8192, 3) f32, new_xyz (8, 2048, 3) f32 -> out (8, 2048, 32)
int32.  Per query row (b, m): first 32 point indices j (ascending) with
|q - p_j|^2 < 0.1^2 under f32 reference rounding, padded with the first
valid index; all-sentinel (8193) when none.

Sharding: data-parallel over batch - core b handles batch b.

Host (per batch): sort points and queries by x.  m-tile t covers sorted
queries [128t, 128t+128); its candidate window is a FIXED slice of sorted
points W_t = [lo_t, lo_t + PCAP) with lo_t on a uniform schedule (same for
every core, so the SPMD program is shared).  Any query whose x +/- 0.1 ball
is not covered by its tile window is recomputed exactly on host (zero rows
for uniform data; correctness never depends on the schedule).

Device (per tile): PE fp32r matmul gives psum = |q-p|^2 - r^2 - EPS over
the window (coords zero-centered on host; EPS = 1.8e-3 superset margin
covers the fp32r deviation, so no reference-valid point is missed).  ACT
Sign (or DVE is_lt on some chunks for load balance) marks in-points; one
DVE uint32-pun not_equal collapses point pairs, a second (on GPSIMD)
collapses pairs to quads; a reversed DVE prefix scan (64-zero pad soaks up
the DVE pipeline-warmup glitch) ranks valid quads clamped at 64; GPSIMD
local_scatter compacts quad ids into 64 slots (descending iteration).

Host decode: each filled slot -> 4 sorted positions -> original indices
via the sort permutation; exact f32 recheck of every candidate; sort by
original index; first 32 + reference padding.  Rows whose quad count hit
the 64 clamp are recomputed exactly (rank is by x-position, not index, so
a clamped row may miss low-index candidates).
"""

import numpy as np

import concourse.bacc as bacc
import concourse.bass as bass
import concourse.mybir as mybir
from concourse import bass_utils
from concourse.tile import TileContext

B, N, M = 8, 8192, 2048
NS = 32
K = 5
NT = M // 128  # 16 m-tiles
PCAP = 3072  # candidate window per tile (sorted points)
CHUNK = 1024  # psum chunk (2 banks)
NCH = PCAP // CHUNK  # 3 chunks per tile
PAIRS = PCAP // 2  # 1536
QUADS = PCAP // 4  # 768
PAD = 64
CLAMP = 64
NSLOT = 66
OFF = 32768
SENT = N + 1
RADIUS2 = np.float32(0.1) * np.float32(0.1)
EPS = np.float32(1.8e-3)
PUN_SIGN = float(0xBF80BF80)  # bf16 (-1,-1) pair: both out (sign path)
PUN_MASK = 0.0  # (0,0) pair: both out (is_lt path)
BIG = 1 << 30
# chunks handled by DVE is_lt instead of ACT sign, for engine balance
DVE_CHUNKS = 5

# fixed window schedule: tile t center ~ N*(128t+64)/M
LOS = []
for _t in range(NT):
    _lo = int(round((N * (128 * _t + 64) / M - PCAP / 2) / 4.0)) * 4
    LOS.append(max(0, min(N - PCAP, _lo)))

_PLAN = {}


def _build():
    if "nc" in _PLAN:
        return _PLAN["nc"]
    f32 = mybir.dt.float32
    f32r = mybir.dt.float32r
    bf16 = mybir.dt.bfloat16
    i16 = mybir.dt.int16
    u32 = mybir.dt.uint32
    Alu = mybir.AluOpType
    Act = mybir.ActivationFunctionType

    nc = bacc.Bacc("TRN2", target_bir_lowering=False)
    qm_t = nc.dram_tensor("qmat", [K, M], f32r, kind="ExternalInput")
    pm_t = nc.dram_tensor("pmat", [K, N], f32r, kind="ExternalInput")
    out_t = nc.dram_tensor("slots", [M, NSLOT], i16, kind="ExternalOutput")

    # global descending quad descriptor: slice at offset (N/4 - lo/4) of
    # this array, read at scan-output position p, yields quad id
    # (lo/4 + QUADS-1-p) + 1 - OFF + N/4 ... chosen so all values are
    # negative int16: descG[i] = N/4 + QUADS - OFF - i; then
    # quad = value + OFF - 1 ... host decodes quad = val + (OFF - 1) where
    # val = descG[N/4 - lo/4 + p] = lo/4 + QUADS - OFF - p + ...  (see
    # decode below: quad_global = slotval + OFF - 1 - N/4 ... simplified:
    # we store val = quad+1-OFF-N/4?  Keep it simple: descG[i] =
    # (N/4 + QUADS - i) - OFF, slice offset o = N/4 - lo/4, so at position
    # p: val = (lo/4 + QUADS - p) - OFF = quad_id + 1 - OFF with
    # quad_id = lo/4 + QUADS - 1 - p  (mirrored window quad).
    NQ4 = N // 4
    descG = (NQ4 + QUADS - np.arange(NQ4 + QUADS, dtype=np.int64) - OFF).astype(
        np.int16
    )
    descG_d = nc.inline_tensor(
        np.ascontiguousarray(np.broadcast_to(descG, (128, NQ4 + QUADS))),
        name="descG",
    )

    with TileContext(nc) as tc:
        with (
            tc.tile_pool(name="const", bufs=1) as cpool,
            tc.tile_pool(name="sgn", bufs=2) as spool,
            tc.tile_pool(name="pmx", bufs=2) as xpool,
            tc.tile_pool(name="qmx", bufs=2) as qpool,
            tc.tile_pool(name="scan", bufs=2) as ipool,
            tc.psum_pool(name="ps", bufs=4) as pp,
        ):
            qt = cpool.tile([K, M], f32r)
            nc.sync.dma_start(qt[:, :], qm_t[:])
            pt = cpool.tile([K, N], f32r)
            nc.sync.dma_start(pt[:, :], pm_t[:])
            descs = cpool.tile([128, NQ4 + QUADS], i16)
            nc.sync.dma_start(descs[:, :], descG_d[:])
            cC = cpool.tile([128, PAD + QUADS], bf16)
            nc.vector.memset(cC, float(CLAMP))

            dsts = cpool.tile([128, NT * NSLOT], i16)

            ci = 0  # global chunk counter for DVE/ACT assignment
            for t in range(NT):
                lo = LOS[t]
                sg = spool.tile([128, PCAP], bf16, tag="sgn")
                for c in range(NCH):
                    ps = pp.tile([128, CHUNK], f32, tag="ps")
                    for s in range(CHUNK // 512):
                        off = lo + c * CHUNK + s * 512
                        nc.tensor.matmul(
                            ps[:, s * 512 : (s + 1) * 512],
                            qt[:, t * 128 : (t + 1) * 128],
                            pt[:, off : off + 512],
                        )
                    seg = sg[:, c * CHUNK : (c + 1) * CHUNK]
                    # interleave a few DVE chunks for ACT/DVE balance
                    if ci % 5 == 2 and ci // 5 < DVE_CHUNKS:
                        nc.vector.tensor_scalar(seg, ps, 0.0, None, Alu.is_lt)
                        # is_lt yields {0,1}: (0,0) pair pun constant; mixing
                        # with sign chunks is fine because pun is per-pair
                        # within one chunk (CHUNK % 4 == 0).
                        pun_src_is_mask = True
                    else:
                        nc.scalar.activation(
                            seg, ps[:, :], Act.Sign, bias=0.0, scale=-1.0
                        )
                        pun_src_is_mask = False
                    # pair pun per chunk (different constants per path)
                    if c == 0:
                        pmx = xpool.tile([128, PAIRS], bf16, tag="pmx")
                    nc.vector.tensor_scalar(
                        pmx[:, c * (CHUNK // 2) : (c + 1) * (CHUNK // 2)],
                        seg.bitcast(u32),
                        PUN_MASK if pun_src_is_mask else PUN_SIGN,
                        None,
                        Alu.not_equal,
                    )
                    ci += 1

                # quad pun (DVE; the op is not in GPSIMD's ucode set)
                qmx = qpool.tile([128, PAD + QUADS], bf16, tag="qmx")
                if t < 2:
                    nc.vector.memset(qmx[:, 0:PAD], 0.0)
                nc.vector.tensor_scalar(
                    qmx[:, PAD:], pmx[:, :].bitcast(u32), 0.0, None, Alu.not_equal
                )

                sc = ipool.tile([128, PAD + QUADS], i16, tag="scan")
                nc.vector.tensor_tensor_scan(
                    sc[:, ::-1], qmx[:, :], cC[:, :], -1.0, Alu.add, Alu.min
                )

                nc.gpsimd.local_scatter(
                    dsts[:, t * NSLOT : (t + 1) * NSLOT],
                    descs[:, NQ4 - lo // 4 : NQ4 - lo // 4 + QUADS],
                    sc[:, 0:QUADS],
                    channels=128,
                    num_elems=NSLOT,
                    num_idxs=QUADS,
                )

            dv = dsts[:, :].rearrange("p (t s) -> p t s", s=NSLOT)
            nc.sync.dma_start(out_t[:].rearrange("(t p) s -> p t s", p=128), dv)

    nc.compile()
    _PLAN["nc"] = nc
    return nc


def _prep(xyz_b, new_b, pperm, qperm):
    half = np.float32(0.5)
    ps = (xyz_b[pperm] - half).astype(np.float32)
    qs = (new_b[qperm] - half).astype(np.float32)
    pmat = np.zeros((K, N), dtype=np.float32)
    pmat[0:3] = ps.T
    pmat[3] = (ps * ps).sum(1, dtype=np.float32)
    pmat[4] = 1.0
    qmat = np.zeros((K, M), dtype=np.float32)
    qmat[0:3] = (np.float32(-2.0) * qs).T
    qmat[3] = 1.0
    qmat[4] = (qs * qs).sum(1, dtype=np.float32) - RADIUS2 - EPS
    return pmat, qmat


def _ref_rows(qrows: np.ndarray, pts: np.ndarray) -> np.ndarray:
    """Exact reference for a set of query rows against all points."""
    d = (qrows[:, None, :] - pts[None, :, :]).astype(np.float32)
    sq = (d * d).astype(np.float32)
    s2 = ((sq[..., 0] + sq[..., 1]) + sq[..., 2]).astype(np.float32)
    nq = qrows.shape[0]
    arange = np.broadcast_to(np.arange(N, dtype=np.int64), (nq, N))
    masked = np.where(s2 < RADIUS2, arange, BIG)
    sv = np.sort(masked, axis=1)[:, :NS]
    vals = np.where(sv >= BIG, SENT, sv)
    first = vals[:, 0:1]
    return np.where(vals == SENT, first, vals)


def kernel(xyz: np.ndarray, new_xyz: np.ndarray) -> np.ndarray:
    xyz = np.ascontiguousarray(np.asarray(xyz, dtype=np.float32))
    new_xyz = np.ascontiguousarray(np.asarray(new_xyz, dtype=np.float32))
    nc = _build()

    pperms = np.empty((B, N), dtype=np.int64)
    qperms = np.empty((B, M), dtype=np.int64)
    in_maps = []
    for b in range(B):
        pperms[b] = np.argsort(xyz[b, :, 0], kind="stable")
        qperms[b] = np.argsort(new_xyz[b, :, 0], kind="stable")
        pmat, qmat = _prep(xyz[b], new_xyz[b], pperms[b], qperms[b])
        in_maps.append({"pmat": pmat, "qmat": qmat})

    res = bass_utils.run_bass_kernel_spmd(nc, in_maps, core_ids=list(range(B)))
    slots = np.stack([res.results[b]["slots"] for b in range(B)], axis=0)

    # decode: slot value -> global quad id (sorted space)
    pool = slots[:, :, :CLAMP].astype(np.int64)
    filled = pool != 0
    quad = np.where(filled, pool + (OFF - 1), 0)  # [B, Msorted, CLAMP]
    spos = (quad[..., None] * 4 + np.arange(4)).reshape(B, M, CLAMP * 4)
    # original point index via sort permutation
    cand = np.take_along_axis(
        np.broadcast_to(pperms[:, None, :], (B, M, N)), spos, axis=2
    )
    bidx = np.arange(B)[:, None, None]
    gat = xyz[bidx, cand, :]  # [B, Msorted, 256, 3]
    q_s = np.take_along_axis(
        new_xyz, np.broadcast_to(qperms[:, :, None], (B, M, 3)), axis=1
    )
    d = (q_s[:, :, None, :] - gat).astype(np.float32)
    sq = (d * d).astype(np.float32)
    s2 = ((sq[..., 0] + sq[..., 1]) + sq[..., 2]).astype(np.float32)
    keepf = np.repeat(filled, 4, axis=2) & (s2 < RADIUS2)

    masked = np.where(keepf, cand, BIG)
    sv = np.sort(masked, axis=2)[:, :, :NS]
    vals = np.where(sv >= BIG, SENT, sv)
    first = vals[:, :, 0:1]
    out_s = np.where(vals == SENT, first, vals)  # sorted-query order

    # fallback 1: quad pool overflow (rank is x-order, may miss low indices)
    trash = slots[:, :, CLAMP] != 0
    # fallback 2: window coverage violation
    for b in range(B):
        px = xyz[b, pperms[b], 0].astype(np.float64)
        qx = new_xyz[b, qperms[b], 0].astype(np.float64)
        bad = trash[b].copy()
        for t in range(NT):
            lo = LOS[t]
            qs = qx[t * 128 : (t + 1) * 128]
            # coverage: every point with x in [q-0.1-eps, q+0.1+eps] must
            # lie inside [lo, lo+PCAP): the nearest excluded points must be
            # strictly outside that x range.
            viol = np.zeros(128, dtype=bool)
            if lo > 0:
                viol |= px[lo - 1] >= qs - (0.1 + 1e-5)
            if lo + PCAP < N:
                viol |= px[lo + PCAP] <= qs + (0.1 + 1e-5)
            bad[t * 128 : (t + 1) * 128] |= viol
        if bad.any():
            rows = np.where(bad)[0]
            out_s[b, rows] = _ref_rows(
                new_xyz[b, qperms[b][rows]], xyz[b]
            )

    # unpermute queries
    out = np.empty_like(out_s)
    for b in range(B):
        out[b, qperms[b]] = out_s[b]
    return out.astype(np.int32)


if __name__ == "__main__":
    rng = np.random.default_rng(0)
    x = rng.random((B, N, 3), dtype=np.float32)
    q = rng.random((B, M, 3), dtype=np.float32)
    o = kernel(x, q)
    print(o.shape, o.dtype)


# revision 9
# speedup vs baseline: 8.5692x; 1.3752x over previous
"""BallQuery Trainium2 kernel, v3: x-sorted spatial windows + PE fp32r
distance matmul + quad-compressed candidate extraction.

Problem: xyz (8, 8192, 3) f32, new_xyz (8, 2048, 3) f32 -> out (8, 2048, 32)
int32.  Per query row (b, m): first 32 point indices j (ascending) with
|q - p_j|^2 < 0.1^2 under f32 reference rounding, padded with the first
valid index; all-sentinel (8193) when none.

Sharding: data-parallel over batch - core b handles batch b.

Host (per batch): sort points and queries by x.  m-tile t covers sorted
queries [128t, 128t+128); its candidate window is a FIXED slice of sorted
points W_t = [lo_t, lo_t + PCAP) with lo_t on a uniform schedule (same for
every core, so the SPMD program is shared).  Any query whose x +/- 0.1 ball
is not covered by its tile window is recomputed exactly on host (zero rows
for uniform data; correctness never depends on the schedule).

Device (per tile): PE fp32r matmul gives psum = |q-p|^2 - r^2 - EPS over
the window (coords zero-centered on host; EPS = 1.8e-3 superset margin
covers the fp32r deviation, so no reference-valid point is missed).  ACT
Sign (or DVE is_lt on some chunks for load balance) marks in-points; one
DVE uint32-pun not_equal collapses point pairs, a second (on GPSIMD)
collapses pairs to quads; a reversed DVE prefix scan (64-zero pad soaks up
the DVE pipeline-warmup glitch) ranks valid quads clamped at 64; GPSIMD
local_scatter compacts quad ids into 64 slots (descending iteration).

Host decode: each filled slot -> 4 sorted positions -> original indices
via the sort permutation; exact f32 recheck of every candidate; sort by
original index; first 32 + reference padding.  Rows whose quad count hit
the 64 clamp are recomputed exactly (rank is by x-position, not index, so
a clamped row may miss low-index candidates).
"""

import numpy as np

import concourse.bacc as bacc
import concourse.bass as bass
import concourse.mybir as mybir
from concourse import bass_utils
from concourse.tile import TileContext

B, N, M = 8, 8192, 2048
NS = 32
K = 5
NT = M // 128  # 16 m-tiles
PCAP = 2560  # candidate window per tile (sorted points)
CHUNKS = (1024, 1024, 512)  # psum chunks (2+2+1 banks)
PAIRS = PCAP // 2  # 1280
QUADS = PCAP // 4  # 640
PAD = 64
CLAMP = 64
NSLOT = 66
OFF = 32768
SENT = N + 1
RADIUS2 = np.float32(0.1) * np.float32(0.1)
EPS = np.float32(1.8e-3)
PUN_SIGN = float(0xBF80BF80)  # bf16 (-1,-1) pair: both out (sign path)
PUN_MASK = 0.0  # (0,0) pair: both out (is_lt path)
BIG = 1 << 30
# chunks handled by DVE is_lt instead of ACT sign, for engine balance
DVE_CHUNKS = 3

# fixed window schedule: tile t center ~ N*(128t+64)/M
LOS = []
for _t in range(NT):
    _lo = int(round((N * (128 * _t + 64) / M - PCAP / 2) / 4.0)) * 4
    LOS.append(max(0, min(N - PCAP, _lo)))

_PLAN = {}


def _build():
    if "nc" in _PLAN:
        return _PLAN["nc"]
    f32 = mybir.dt.float32
    f32r = mybir.dt.float32r
    bf16 = mybir.dt.bfloat16
    i16 = mybir.dt.int16
    u32 = mybir.dt.uint32
    Alu = mybir.AluOpType
    Act = mybir.ActivationFunctionType

    nc = bacc.Bacc("TRN2", target_bir_lowering=False)
    qm_t = nc.dram_tensor("qmat", [K, M], f32r, kind="ExternalInput")
    pm_t = nc.dram_tensor("pmat", [K, N], f32r, kind="ExternalInput")
    out_t = nc.dram_tensor("slots", [M, NSLOT], i16, kind="ExternalOutput")

    # global descending quad descriptor: slice at offset (N/4 - lo/4) of
    # this array, read at scan-output position p, yields quad id
    # (lo/4 + QUADS-1-p) + 1 - OFF + N/4 ... chosen so all values are
    # negative int16: descG[i] = N/4 + QUADS - OFF - i; then
    # quad = value + OFF - 1 ... host decodes quad = val + (OFF - 1) where
    # val = descG[N/4 - lo/4 + p] = lo/4 + QUADS - OFF - p + ...  (see
    # decode below: quad_global = slotval + OFF - 1 - N/4 ... simplified:
    # we store val = quad+1-OFF-N/4?  Keep it simple: descG[i] =
    # (N/4 + QUADS - i) - OFF, slice offset o = N/4 - lo/4, so at position
    # p: val = (lo/4 + QUADS - p) - OFF = quad_id + 1 - OFF with
    # quad_id = lo/4 + QUADS - 1 - p  (mirrored window quad).
    NQ4 = N // 4
    descG = (NQ4 + QUADS - np.arange(NQ4 + QUADS, dtype=np.int64) - OFF).astype(
        np.int16
    )
    descG_d = nc.inline_tensor(
        np.ascontiguousarray(np.broadcast_to(descG, (128, NQ4 + QUADS))),
        name="descG",
    )

    with TileContext(nc) as tc:
        with (
            tc.tile_pool(name="const", bufs=1) as cpool,
            tc.tile_pool(name="sgn", bufs=2) as spool,
            tc.tile_pool(name="pmx", bufs=2) as xpool,
            tc.tile_pool(name="qmx", bufs=2) as qpool,
            tc.tile_pool(name="scan", bufs=2) as ipool,
            tc.psum_pool(name="psA", bufs=3) as ppA,
            tc.psum_pool(name="psB", bufs=2) as ppB,
        ):
            qt = cpool.tile([K, M], f32r)
            for i in range(4):
                nc.sync.dma_start(
                    qt[:, i * 512 : (i + 1) * 512], qm_t[:, i * 512 : (i + 1) * 512]
                )
            # pmat uses only K=5 partitions: split across DMA queues so the
            # per-partition byte cost parallelizes instead of serializing.
            pt = cpool.tile([K, N], f32r)
            for i in range(8):
                nc.sync.dma_start(
                    pt[:, i * 1024 : (i + 1) * 1024],
                    pm_t[:, i * 1024 : (i + 1) * 1024],
                )
            descs = cpool.tile([128, NQ4 + QUADS], i16)
            half_d = (NQ4 + QUADS) // 2
            for i in range(2):
                nc.sync.dma_start(
                    descs[:, i * half_d : (i + 1) * half_d],
                    descG_d[:, i * half_d : (i + 1) * half_d],
                )
            cC = cpool.tile([128, PAD + QUADS], bf16)
            nc.vector.memset(cC, float(CLAMP))

            dsts = cpool.tile([128, NT * NSLOT], i16)

            ci = 0  # global [1024]-chunk counter for DVE/ACT assignment
            for t in range(NT):
                lo = LOS[t]
                sg = spool.tile([128, PCAP], bf16, tag="sgn")
                coff = 0
                for c, csz in enumerate(CHUNKS):
                    pool = ppA if csz == 1024 else ppB
                    ps = pool.tile([128, csz], f32, tag="ps")
                    for s in range(csz // 512):
                        off = lo + coff + s * 512
                        nc.tensor.matmul(
                            ps[:, s * 512 : (s + 1) * 512],
                            qt[:, t * 128 : (t + 1) * 128],
                            pt[:, off : off + 512],
                        )
                    seg = sg[:, coff : coff + csz]
                    # interleave a few DVE chunks for ACT/DVE balance
                    if csz == 1024 and ci % 11 == 5 and ci // 11 < DVE_CHUNKS:
                        nc.vector.tensor_scalar(seg, ps, 0.0, None, Alu.is_lt)
                        pun_c = PUN_MASK
                    else:
                        nc.scalar.activation(
                            seg, ps[:, :], Act.Sign, bias=0.0, scale=-1.0
                        )
                        pun_c = PUN_SIGN
                    if csz == 1024:
                        ci += 1
                    # pair pun per chunk (constant differs per path)
                    if c == 0:
                        pmx = xpool.tile([128, PAIRS], bf16, tag="pmx")
                    nc.vector.tensor_scalar(
                        pmx[:, coff // 2 : (coff + csz) // 2],
                        seg.bitcast(u32),
                        pun_c,
                        None,
                        Alu.not_equal,
                    )
                    coff += csz

                # quad pun (DVE; the op is not in GPSIMD's ucode set)
                qmx = qpool.tile([128, PAD + QUADS], bf16, tag="qmx")
                if t < 2:
                    nc.vector.memset(qmx[:, 0:PAD], 0.0)
                nc.vector.tensor_scalar(
                    qmx[:, PAD:], pmx[:, :].bitcast(u32), 0.0, None, Alu.not_equal
                )

                sc = ipool.tile([128, PAD + QUADS], i16, tag="scan")
                nc.vector.tensor_tensor_scan(
                    sc[:, ::-1], qmx[:, :], cC[:, :], -1.0, Alu.add, Alu.min
                )

                nc.gpsimd.local_scatter(
                    dsts[:, t * NSLOT : (t + 1) * NSLOT],
                    descs[:, NQ4 - lo // 4 : NQ4 - lo // 4 + QUADS],
                    sc[:, 0:QUADS],
                    channels=128,
                    num_elems=NSLOT,
                    num_idxs=QUADS,
                )
                # stream the output per 4-tile group to shorten the tail
                if t % 4 == 3:
                    g = t - 3
                    dv = dsts[:, g * NSLOT : (t + 1) * NSLOT].rearrange(
                        "p (t s) -> p t s", s=NSLOT
                    )
                    nc.sync.dma_start(
                        out_t[:]
                        .rearrange("(t p) s -> p t s", p=128)[:, g : t + 1, :],
                        dv,
                    )

    nc.compile()
    _PLAN["nc"] = nc
    return nc


def _prep(xyz_b, new_b, pperm, qperm):
    half = np.float32(0.5)
    ps = (xyz_b[pperm] - half).astype(np.float32)
    qs = (new_b[qperm] - half).astype(np.float32)
    pmat = np.zeros((K, N), dtype=np.float32)
    pmat[0:3] = ps.T
    pmat[3] = (ps * ps).sum(1, dtype=np.float32)
    pmat[4] = 1.0
    qmat = np.zeros((K, M), dtype=np.float32)
    qmat[0:3] = (np.float32(-2.0) * qs).T
    qmat[3] = 1.0
    qmat[4] = (qs * qs).sum(1, dtype=np.float32) - RADIUS2 - EPS
    return pmat, qmat


def _ref_rows(qrows: np.ndarray, pts: np.ndarray) -> np.ndarray:
    """Exact reference for a set of query rows against all points."""
    d = (qrows[:, None, :] - pts[None, :, :]).astype(np.float32)
    sq = (d * d).astype(np.float32)
    s2 = ((sq[..., 0] + sq[..., 1]) + sq[..., 2]).astype(np.float32)
    nq = qrows.shape[0]
    arange = np.broadcast_to(np.arange(N, dtype=np.int64), (nq, N))
    masked = np.where(s2 < RADIUS2, arange, BIG)
    sv = np.sort(masked, axis=1)[:, :NS]
    vals = np.where(sv >= BIG, SENT, sv)
    first = vals[:, 0:1]
    return np.where(vals == SENT, first, vals)


def kernel(xyz: np.ndarray, new_xyz: np.ndarray) -> np.ndarray:
    xyz = np.ascontiguousarray(np.asarray(xyz, dtype=np.float32))
    new_xyz = np.ascontiguousarray(np.asarray(new_xyz, dtype=np.float32))
    nc = _build()

    pperms = np.empty((B, N), dtype=np.int64)
    qperms = np.empty((B, M), dtype=np.int64)
    in_maps = []
    for b in range(B):
        pperms[b] = np.argsort(xyz[b, :, 0], kind="stable")
        qperms[b] = np.argsort(new_xyz[b, :, 0], kind="stable")
        pmat, qmat = _prep(xyz[b], new_xyz[b], pperms[b], qperms[b])
        in_maps.append({"pmat": pmat, "qmat": qmat})

    res = bass_utils.run_bass_kernel_spmd(nc, in_maps, core_ids=list(range(B)))
    slots = np.stack([res.results[b]["slots"] for b in range(B)], axis=0)

    # decode: slot value -> global quad id (sorted space)
    pool = slots[:, :, :CLAMP].astype(np.int64)
    filled = pool != 0
    quad = np.where(filled, pool + (OFF - 1), 0)  # [B, Msorted, CLAMP]
    spos = (quad[..., None] * 4 + np.arange(4)).reshape(B, M, CLAMP * 4)
    # original point index via sort permutation
    cand = np.take_along_axis(
        np.broadcast_to(pperms[:, None, :], (B, M, N)), spos, axis=2
    )
    bidx = np.arange(B)[:, None, None]
    gat = xyz[bidx, cand, :]  # [B, Msorted, 256, 3]
    q_s = np.take_along_axis(
        new_xyz, np.broadcast_to(qperms[:, :, None], (B, M, 3)), axis=1
    )
    d = (q_s[:, :, None, :] - gat).astype(np.float32)
    sq = (d * d).astype(np.float32)
    s2 = ((sq[..., 0] + sq[..., 1]) + sq[..., 2]).astype(np.float32)
    keepf = np.repeat(filled, 4, axis=2) & (s2 < RADIUS2)

    masked = np.where(keepf, cand, BIG)
    sv = np.sort(masked, axis=2)[:, :, :NS]
    vals = np.where(sv >= BIG, SENT, sv)
    first = vals[:, :, 0:1]
    out_s = np.where(vals == SENT, first, vals)  # sorted-query order

    # fallback 1: quad pool overflow (rank is x-order, may miss low indices)
    trash = slots[:, :, CLAMP] != 0
    # fallback 2: window coverage violation
    for b in range(B):
        px = xyz[b, pperms[b], 0].astype(np.float64)
        qx = new_xyz[b, qperms[b], 0].astype(np.float64)
        bad = trash[b].copy()
        for t in range(NT):
            lo = LOS[t]
            qs = qx[t * 128 : (t + 1) * 128]
            # coverage: every point with x in [q-0.1-eps, q+0.1+eps] must
            # lie inside [lo, lo+PCAP): the nearest excluded points must be
            # strictly outside that x range.
            viol = np.zeros(128, dtype=bool)
            if lo > 0:
                viol |= px[lo - 1] >= qs - (0.1 + 1e-5)
            if lo + PCAP < N:
                viol |= px[lo + PCAP] <= qs + (0.1 + 1e-5)
            bad[t * 128 : (t + 1) * 128] |= viol
        if bad.any():
            rows = np.where(bad)[0]
            out_s[b, rows] = _ref_rows(
                new_xyz[b, qperms[b][rows]], xyz[b]
            )

    # unpermute queries
    out = np.empty_like(out_s)
    for b in range(B):
        out[b, qperms[b]] = out_s[b]
    return out.astype(np.int32)


if __name__ == "__main__":
    rng = np.random.default_rng(0)
    x = rng.random((B, N, 3), dtype=np.float32)
    q = rng.random((B, M, 3), dtype=np.float32)
    o = kernel(x, q)
    print(o.shape, o.dtype)


# revision 13
# speedup vs baseline: 8.6992x; 1.0152x over previous
"""BallQuery Trainium2 kernel, v3: x-sorted spatial windows + PE fp32r
distance matmul + quad-compressed candidate extraction.

Problem: xyz (8, 8192, 3) f32, new_xyz (8, 2048, 3) f32 -> out (8, 2048, 32)
int32.  Per query row (b, m): first 32 point indices j (ascending) with
|q - p_j|^2 < 0.1^2 under f32 reference rounding, padded with the first
valid index; all-sentinel (8193) when none.

Sharding: data-parallel over batch - core b handles batch b.

Host (per batch): sort points and queries by x.  m-tile t covers sorted
queries [128t, 128t+128); its candidate window is a FIXED slice of sorted
points W_t = [lo_t, lo_t + PCAP) with lo_t on a uniform schedule (same for
every core, so the SPMD program is shared).  Any query whose x +/- 0.1 ball
is not covered by its tile window is recomputed exactly on host (zero rows
for uniform data; correctness never depends on the schedule).

Device (per tile): PE fp32r matmul gives psum = |q-p|^2 - r^2 - EPS over
the window (coords zero-centered on host; EPS = 1.8e-3 superset margin
covers the fp32r deviation, so no reference-valid point is missed).  ACT
Sign (or DVE is_lt on some chunks for load balance) marks in-points; one
DVE uint32-pun not_equal collapses point pairs, a second (on GPSIMD)
collapses pairs to quads; a reversed DVE prefix scan (64-zero pad soaks up
the DVE pipeline-warmup glitch) ranks valid quads clamped at 64; GPSIMD
local_scatter compacts quad ids into 64 slots (descending iteration).

Host decode: each filled slot -> 4 sorted positions -> original indices
via the sort permutation; exact f32 recheck of every candidate; sort by
original index; first 32 + reference padding.  Rows whose quad count hit
the 64 clamp are recomputed exactly (rank is by x-position, not index, so
a clamped row may miss low-index candidates).
"""

import numpy as np

import concourse.bacc as bacc
import concourse.bass as bass
import concourse.mybir as mybir
from concourse import bass_utils
from concourse.tile import TileContext

B, N, M = 8, 8192, 2048
NS = 32
K = 5
NT = M // 128  # 16 m-tiles
PCAP = 2560  # candidate window per tile (sorted points)
CHUNKS = (1024, 1024, 512)  # psum chunks (2+2+1 banks)
PAIRS = PCAP // 2  # 1280
QUADS = PCAP // 4  # 640
PAD = 64
CLAMP = 64
NSLOT = 66
OFF = 32768
SENT = N + 1
RADIUS2 = np.float32(0.1) * np.float32(0.1)
EPS = np.float32(1.8e-3)
PUN_SIGN = float(0xBF80BF80)  # bf16 (-1,-1) pair: both out (sign path)
PUN_MASK = 0.0  # (0,0) pair: both out (is_lt path)
BIG = 1 << 30
# chunks handled by DVE is_lt instead of ACT sign, for engine balance
DVE_CHUNKS = 5

# fixed window schedule: tile t center ~ N*(128t+64)/M
LOS = []
for _t in range(NT):
    _lo = int(round((N * (128 * _t + 64) / M - PCAP / 2) / 4.0)) * 4
    LOS.append(max(0, min(N - PCAP, _lo)))

_PLAN = {}


def _build():
    if "nc" in _PLAN:
        return _PLAN["nc"]
    f32 = mybir.dt.float32
    f32r = mybir.dt.float32r
    bf16 = mybir.dt.bfloat16
    i16 = mybir.dt.int16
    u32 = mybir.dt.uint32
    Alu = mybir.AluOpType
    Act = mybir.ActivationFunctionType

    nc = bacc.Bacc("TRN2", target_bir_lowering=False)
    qm_t = nc.dram_tensor("qmat", [K, M], f32r, kind="ExternalInput")
    pm_t = nc.dram_tensor("pmat", [K, N], f32r, kind="ExternalInput")
    out_t = nc.dram_tensor("slots", [M, NSLOT], i16, kind="ExternalOutput")

    # global descending quad descriptor: slice at offset (N/4 - lo/4) of
    # this array, read at scan-output position p, yields quad id
    # (lo/4 + QUADS-1-p) + 1 - OFF + N/4 ... chosen so all values are
    # negative int16: descG[i] = N/4 + QUADS - OFF - i; then
    # quad = value + OFF - 1 ... host decodes quad = val + (OFF - 1) where
    # val = descG[N/4 - lo/4 + p] = lo/4 + QUADS - OFF - p + ...  (see
    # decode below: quad_global = slotval + OFF - 1 - N/4 ... simplified:
    # we store val = quad+1-OFF-N/4?  Keep it simple: descG[i] =
    # (N/4 + QUADS - i) - OFF, slice offset o = N/4 - lo/4, so at position
    # p: val = (lo/4 + QUADS - p) - OFF = quad_id + 1 - OFF with
    # quad_id = lo/4 + QUADS - 1 - p  (mirrored window quad).
    NQ4 = N // 4
    descG = (NQ4 + QUADS - np.arange(NQ4 + QUADS, dtype=np.int64) - OFF).astype(
        np.int16
    )
    descG_d = nc.inline_tensor(
        np.ascontiguousarray(np.broadcast_to(descG, (128, NQ4 + QUADS))),
        name="descG",
    )

    with TileContext(nc) as tc:
        with (
            tc.tile_pool(name="const", bufs=1) as cpool,
            tc.tile_pool(name="sgn", bufs=2) as spool,
            tc.tile_pool(name="pmx", bufs=2) as xpool,
            tc.tile_pool(name="qmx", bufs=2) as qpool,
            tc.tile_pool(name="scan", bufs=2) as ipool,
            tc.psum_pool(name="psA", bufs=3) as ppA,
            tc.psum_pool(name="psB", bufs=2) as ppB,
        ):
            # pmat/qmat use only K=5 partitions: split across DMA queues so
            # the per-partition byte cost parallelizes; issue the slices the
            # first tiles need before the rest so compute starts early.
            qt = cpool.tile([K, M], f32r)
            pt = cpool.tile([K, N], f32r)
            nc.sync.dma_start(qt[:, 0:256], qm_t[:, 0:256])
            for i in range(5):
                nc.sync.dma_start(
                    pt[:, i * 768 : (i + 1) * 768], pm_t[:, i * 768 : (i + 1) * 768]
                )
            for i in range(1, 8):
                nc.sync.dma_start(
                    qt[:, i * 256 : (i + 1) * 256], qm_t[:, i * 256 : (i + 1) * 256]
                )
            for i in range(5, 10):
                lo_i, hi_i = i * 768, min((i + 1) * 768, N)
                nc.sync.dma_start(pt[:, lo_i:hi_i], pm_t[:, lo_i:hi_i])
            descs = cpool.tile([128, NQ4 + QUADS], i16)
            half_d = (NQ4 + QUADS) // 2
            for i in range(2):
                nc.sync.dma_start(
                    descs[:, i * half_d : (i + 1) * half_d],
                    descG_d[:, i * half_d : (i + 1) * half_d],
                )
            cC = cpool.tile([128, PAD + QUADS], bf16)
            nc.vector.memset(cC, float(CLAMP))

            dsts = cpool.tile([128, NT * NSLOT], i16)

            ci = 0  # global [1024]-chunk counter for DVE/ACT assignment
            for t in range(NT):
                lo = LOS[t]
                sg = spool.tile([128, PCAP], bf16, tag="sgn")
                coff = 0
                for c, csz in enumerate(CHUNKS):
                    pool = ppA if csz == 1024 else ppB
                    ps = pool.tile([128, csz], f32, tag="ps")
                    for s in range(csz // 512):
                        off = lo + coff + s * 512
                        nc.tensor.matmul(
                            ps[:, s * 512 : (s + 1) * 512],
                            qt[:, t * 128 : (t + 1) * 128],
                            pt[:, off : off + 512],
                        )
                    seg = sg[:, coff : coff + csz]
                    # interleave a few DVE chunks for ACT/DVE balance
                    if csz == 1024 and ci % 6 == 3 and ci // 6 < DVE_CHUNKS:
                        nc.vector.tensor_scalar(seg, ps, 0.0, None, Alu.is_lt)
                        pun_c = PUN_MASK
                    else:
                        nc.scalar.activation(
                            seg, ps[:, :], Act.Sign, bias=0.0, scale=-1.0
                        )
                        pun_c = PUN_SIGN
                    if csz == 1024:
                        ci += 1
                    # pair pun per chunk (constant differs per path)
                    if c == 0:
                        pmx = xpool.tile([128, PAIRS], bf16, tag="pmx")
                    nc.vector.tensor_scalar(
                        pmx[:, coff // 2 : (coff + csz) // 2],
                        seg.bitcast(u32),
                        pun_c,
                        None,
                        Alu.not_equal,
                    )
                    coff += csz

                # quad pun (DVE; the op is not in GPSIMD's ucode set)
                qmx = qpool.tile([128, PAD + QUADS], bf16, tag="qmx")
                if t < 2:
                    nc.vector.memset(qmx[:, 0:PAD], 0.0)
                nc.vector.tensor_scalar(
                    qmx[:, PAD:], pmx[:, :].bitcast(u32), 0.0, None, Alu.not_equal
                )

                sc = ipool.tile([128, PAD + QUADS], i16, tag="scan")
                nc.vector.tensor_tensor_scan(
                    sc[:, ::-1], qmx[:, :], cC[:, :], -1.0, Alu.add, Alu.min
                )

                nc.gpsimd.local_scatter(
                    dsts[:, t * NSLOT : (t + 1) * NSLOT],
                    descs[:, NQ4 - lo // 4 : NQ4 - lo // 4 + QUADS],
                    sc[:, 0:QUADS],
                    channels=128,
                    num_elems=NSLOT,
                    num_idxs=QUADS,
                )
                # stream the output to shorten the tail (finer at the end)
                if t in (3, 7, 11, 13, 14, 15):
                    g = {3: 0, 7: 4, 11: 8, 13: 12, 14: 14, 15: 15}[t]
                    dv = dsts[:, g * NSLOT : (t + 1) * NSLOT].rearrange(
                        "p (t s) -> p t s", s=NSLOT
                    )
                    nc.sync.dma_start(
                        out_t[:]
                        .rearrange("(t p) s -> p t s", p=128)[:, g : t + 1, :],
                        dv,
                    )

    nc.compile()
    _PLAN["nc"] = nc
    return nc


def _prep(xyz_b, new_b, pperm, qperm):
    half = np.float32(0.5)
    ps = (xyz_b[pperm] - half).astype(np.float32)
    qs = (new_b[qperm] - half).astype(np.float32)
    pmat = np.zeros((K, N), dtype=np.float32)
    pmat[0:3] = ps.T
    pmat[3] = (ps * ps).sum(1, dtype=np.float32)
    pmat[4] = 1.0
    qmat = np.zeros((K, M), dtype=np.float32)
    qmat[0:3] = (np.float32(-2.0) * qs).T
    qmat[3] = 1.0
    qmat[4] = (qs * qs).sum(1, dtype=np.float32) - RADIUS2 - EPS
    return pmat, qmat


def _ref_rows(qrows: np.ndarray, pts: np.ndarray) -> np.ndarray:
    """Exact reference for a set of query rows against all points."""
    d = (qrows[:, None, :] - pts[None, :, :]).astype(np.float32)
    sq = (d * d).astype(np.float32)
    s2 = ((sq[..., 0] + sq[..., 1]) + sq[..., 2]).astype(np.float32)
    nq = qrows.shape[0]
    arange = np.broadcast_to(np.arange(N, dtype=np.int64), (nq, N))
    masked = np.where(s2 < RADIUS2, arange, BIG)
    sv = np.sort(masked, axis=1)[:, :NS]
    vals = np.where(sv >= BIG, SENT, sv)
    first = vals[:, 0:1]
    return np.where(vals == SENT, first, vals)


def kernel(xyz: np.ndarray, new_xyz: np.ndarray) -> np.ndarray:
    xyz = np.ascontiguousarray(np.asarray(xyz, dtype=np.float32))
    new_xyz = np.ascontiguousarray(np.asarray(new_xyz, dtype=np.float32))
    nc = _build()

    pperms = np.empty((B, N), dtype=np.int64)
    qperms = np.empty((B, M), dtype=np.int64)
    in_maps = []
    for b in range(B):
        pperms[b] = np.argsort(xyz[b, :, 0], kind="stable")
        qperms[b] = np.argsort(new_xyz[b, :, 0], kind="stable")
        pmat, qmat = _prep(xyz[b], new_xyz[b], pperms[b], qperms[b])
        in_maps.append({"pmat": pmat, "qmat": qmat})

    res = bass_utils.run_bass_kernel_spmd(nc, in_maps, core_ids=list(range(B)))
    slots = np.stack([res.results[b]["slots"] for b in range(B)], axis=0)

    # decode: slot value -> global quad id (sorted space)
    pool = slots[:, :, :CLAMP].astype(np.int64)
    filled = pool != 0
    quad = np.where(filled, pool + (OFF - 1), 0)  # [B, Msorted, CLAMP]
    spos = (quad[..., None] * 4 + np.arange(4)).reshape(B, M, CLAMP * 4)
    # original point index via sort permutation
    cand = np.take_along_axis(
        np.broadcast_to(pperms[:, None, :], (B, M, N)), spos, axis=2
    )
    bidx = np.arange(B)[:, None, None]
    gat = xyz[bidx, cand, :]  # [B, Msorted, 256, 3]
    q_s = np.take_along_axis(
        new_xyz, np.broadcast_to(qperms[:, :, None], (B, M, 3)), axis=1
    )
    d = (q_s[:, :, None, :] - gat).astype(np.float32)
    sq = (d * d).astype(np.float32)
    s2 = ((sq[..., 0] + sq[..., 1]) + sq[..., 2]).astype(np.float32)
    keepf = np.repeat(filled, 4, axis=2) & (s2 < RADIUS2)

    masked = np.where(keepf, cand, BIG)
    sv = np.sort(masked, axis=2)[:, :, :NS]
    vals = np.where(sv >= BIG, SENT, sv)
    first = vals[:, :, 0:1]
    out_s = np.where(vals == SENT, first, vals)  # sorted-query order

    # fallback 1: quad pool overflow (rank is x-order, may miss low indices)
    trash = slots[:, :, CLAMP] != 0
    # fallback 2: window coverage violation
    for b in range(B):
        px = xyz[b, pperms[b], 0].astype(np.float64)
        qx = new_xyz[b, qperms[b], 0].astype(np.float64)
        bad = trash[b].copy()
        for t in range(NT):
            lo = LOS[t]
            qs = qx[t * 128 : (t + 1) * 128]
            # coverage: every point with x in [q-0.1-eps, q+0.1+eps] must
            # lie inside [lo, lo+PCAP): the nearest excluded points must be
            # strictly outside that x range.
            viol = np.zeros(128, dtype=bool)
            if lo > 0:
                viol |= px[lo - 1] >= qs - (0.1 + 1e-5)
            if lo + PCAP < N:
                viol |= px[lo + PCAP] <= qs + (0.1 + 1e-5)
            bad[t * 128 : (t + 1) * 128] |= viol
        if bad.any():
            rows = np.where(bad)[0]
            out_s[b, rows] = _ref_rows(
                new_xyz[b, qperms[b][rows]], xyz[b]
            )

    # unpermute queries
    out = np.empty_like(out_s)
    for b in range(B):
        out[b, qperms[b]] = out_s[b]
    return out.astype(np.int32)


if __name__ == "__main__":
    rng = np.random.default_rng(0)
    x = rng.random((B, N, 3), dtype=np.float32)
    q = rng.random((B, M, 3), dtype=np.float32)
    o = kernel(x, q)
    print(o.shape, o.dtype)
